# revision 16
# baseline (speedup 1.0000x reference)
"""BCM_Conv2d_fft kernel for Trainium2 (8 NeuronCores, batch-parallel).

The reference is a block-circulant 3x3 conv computed via per-block
rfft/irfft over the channel-block axis (block size 8). Per-frequency the
block products are independent, so in a real-DFT channel basis the
256->256 channel mixing matrix of each conv tap is block-diagonal with
frequency groups {f0:32, f4:32, f1:64, f2:64, f3:64}. Grouping
{f0,f4,f1} -> chunk0 and {f2,f3} -> chunk1 makes every tap's mixing
matrix chunk-diagonal: the conv needs 9 matmuls per output tile per
chunk instead of 18 - half the direct-conv PE work.

Device pipeline per core (one image):
  1. fwd:  xhat = A @ x      per pixel    (A = real-DFT, freq-major rows)
  2. conv: ohat = sum_pos M_pos @ shift(xhat)   (chunk-diagonal M)
  3. inv:  out  = Ainv @ ohat + b

All matmuls run in bf16 (measured rel err ~3.1e-3, limit 2e-2): fp32
weights can't use fast-weight-load, so each fp32 LDWEIGHTS takes ~225ns
and gates the matmul cadence at ~252ns, while bf16 weights hide under
the ~190ns N=448 stream. x is padded AND cast to bf16 on the host,
halving its DMA bytes; xhat/ohat are cast to bf16 in the PSUM->SBUF
copies that must happen anyway. The fwd transform covers interior
pixels only (padded borders are zero: their xhat rows/cols are memset).

The first ~10us are DMA-bound (x + weights stream in at ~0.3MB/us over
three queues), so the tensor engine is fed no-dependency dummy matmuls
on a zeroed tile - an initial burst plus fillers between fwd tiles -
which bridge data gaps and keep the HAM clock-gate released (2.4GHz
instead of the 1.2GHz cold state). DMA issue order gives each queue its
first-needed bytes first (x split into 5 row-pieces per chunk, conv
weights into tap-order pieces). All 7 fwd tiles are emitted before the
first conv tile so a weight-blocked conv can never starve ready fwd
work in the tensor FIFO. Outputs alternate between two DMA queues; the
last tile runs as two half-tiles so the final writeback drains early.

Sharding: batch B=8 -> one image per core.
"""

import os

import ml_dtypes
import numpy as np

import concourse.bacc as bacc
import concourse.mybir as mybir
import concourse.tile as tile
from concourse.bass import ts
from concourse.bass_utils import run_bass_kernel_spmd

N_CORES = 8
C = 256
H = W = 56
HP = H + 2
KK = 3
BS = 8
L = H * W
RPT = 8                  # output rows per tile
NT = RPT * W             # 448 pixels per tile
N_TILES = L // NT        # 7
MCH = C // 128           # 2 channel chunks

F32 = mybir.dt.float32
BF16 = mybir.dt.bfloat16

# weight block column indices in the packed wb tensor [128, 26*128] bf16
FWD_BLK = lambda i, c: i * MCH + c            # i = in chunk, c = out chunk
CONV_BLK = lambda pos, c: 4 + pos * MCH + c
INV_BLK = lambda k, m: 22 + k * MCH + m
N_BLKS = 26
N_WARMUP = 10            # dummy matmuls to release the HAM clock gate

LAST_RESULT = None


def _freq_matrices(w: np.ndarray):
    """Build A [256,256], Ms (9x [256,256] chunk-diag), Ainv from w."""
    F = np.zeros((8, 8))
    FI = np.fft.rfft(np.eye(8), axis=-1)
    F[0] = FI[:, 0].real
    F[1], F[2] = FI[:, 1].real, FI[:, 1].imag
    F[3], F[4] = FI[:, 2].real, FI[:, 2].imag
    F[5], F[6] = FI[:, 3].real, FI[:, 3].imag
    F[7] = FI[:, 4].real

    def fm(bk, comp):
        if comp == 0:
            return bk
        if comp == 7:
            return 32 + bk
        if comp in (1, 2):
            return 64 + 2 * bk + (comp - 1)
        if comp in (3, 4):
            return 128 + 2 * bk + (comp - 3)
        return 192 + 2 * bk + (comp - 5)

    A = np.zeros((256, 256))
    for bk in range(32):
        for comp in range(8):
            A[fm(bk, comp), bk * 8:(bk + 1) * 8] = F[comp]
    Ainv = np.linalg.inv(A)

    wf = np.fft.rfft(w.astype(np.float64), axis=-1)  # [32, 288, 5]
    Ms = []
    for pos in range(9):
        M = np.zeros((256, 256))
        for pb in range(32):
            for kb in range(32):
                kc = pos * 32 + kb
                M[fm(pb, 0), fm(kb, 0)] += wf[pb, kc, 0].real
                M[fm(pb, 7), fm(kb, 7)] += wf[pb, kc, 4].real
                for fi in range(3):
                    re_i, im_i = 1 + 2 * fi, 2 + 2 * fi
                    Wr, Wi = wf[pb, kc, fi + 1].real, wf[pb, kc, fi + 1].imag
                    M[fm(pb, re_i), fm(kb, re_i)] += Wr
                    M[fm(pb, re_i), fm(kb, im_i)] += -Wi
                    M[fm(pb, im_i), fm(kb, re_i)] += Wi
                    M[fm(pb, im_i), fm(kb, im_i)] += Wr
        Ms.append(M)
    return A, Ms, Ainv


def _pack_weights(w: np.ndarray) -> np.ndarray:
    """-> [128, 26*128] bf16: lhsT blocks for fwd, conv, inv stages."""
    A, Ms, Ainv = _freq_matrices(w)
    wb = np.zeros((128, N_BLKS * 128), np.float32)

    sl = lambda i: slice(i * 128, (i + 1) * 128)
    for i in range(MCH):
        for c in range(MCH):
            wb[:, sl(FWD_BLK(i, c))] = A[sl(c), sl(i)].T
    for pos in range(9):
        for c in range(MCH):
            wb[:, sl(CONV_BLK(pos, c))] = Ms[pos][sl(c), sl(c)].T
    for k in range(MCH):
        for m in range(MCH):
            wb[:, sl(INV_BLK(k, m))] = Ainv[sl(m), sl(k)].T
    return wb.astype(ml_dtypes.bfloat16)


def _kernel_body(tc, x, wb, bias, out):
    nc = tc.nc
    with (
        tc.tile_pool(name="const", bufs=1) as const_pool,
        tc.tile_pool(name="xb", bufs=1) as xb_pool,
        tc.tile_pool(name="xh", bufs=1) as xh_pool,
        tc.tile_pool(name="oh", bufs=6) as oh_pool,
        tc.tile_pool(name="ob", bufs=4) as ob_pool,
        tc.tile_pool(name="psf", bufs=3, space="PSUM") as psf_pool,
        tc.tile_pool(name="psc", bufs=3, space="PSUM") as psc_pool,
        tc.tile_pool(name="psi", bufs=2, space="PSUM") as psi_pool,
    ):
        # PE warm-up source: zeroed on the (idle) DVE before any DMA issue
        warm = const_pool.tile([128, RPT * HP], BF16)
        nc.vector.memset(warm[:], 0)

        # --- DMA schedule -------------------------------------------------
        # Three issuing engines feed three hardware queues; each queue gets
        # its first-needed bytes first:
        #   sync:   x c0 rows 0-8, bias, x c0 rest, conv weights pos 6-8
        #   gpsimd: x c1 rows 0-8, x c1 rest, inv weights (outputs later)
        #   scalar: fwd weights, conv weights pos 0-5
        wb_sb = const_pool.tile([128, N_BLKS * 128], BF16)
        blk = lambda idx: wb_sb[:, ts(idx, 128)]

        xbt = []
        for i in range(MCH):
            xb_t = xb_pool.tile([128, HP * HP], BF16, tag=f"xb{i}")
            xbt.append(xb_t)
        bias_sb = const_pool.tile([128, MCH], F32)

        row_splits = [0, 10, 20, 30, 44, HP]
        eng = [nc.sync, nc.gpsimd]
        for i in range(MCH):
            r0, r1 = row_splits[0], row_splits[1]
            eng[i].dma_start(
                out=xbt[i][:, r0 * HP:r1 * HP],
                in_=x[ts(i, 128), r0:r1, :].rearrange("p h w -> p (h w)"),
            )
        nc.scalar.dma_start(out=wb_sb[:, 0:4 * 128], in_=wb[:, 0:4 * 128])
        nc.scalar.dma_start(out=bias_sb[:], in_=bias[:, :])
        for r0, r1 in zip(row_splits[1:-1], row_splits[2:]):
            for i in range(MCH):
                eng[i].dma_start(
                    out=xbt[i][:, r0 * HP:r1 * HP],
                    in_=x[ts(i, 128), r0:r1, :].rearrange("p h w -> p (h w)"),
                )
        nc.scalar.dma_start(out=wb_sb[:, 4 * 128:10 * 128],
                            in_=wb[:, 4 * 128:10 * 128])
        nc.scalar.dma_start(out=wb_sb[:, 10 * 128:16 * 128],
                            in_=wb[:, 10 * 128:16 * 128])
        nc.scalar.dma_start(out=wb_sb[:, 16 * 128:22 * 128],
                            in_=wb[:, 16 * 128:22 * 128])
        nc.scalar.dma_start(out=wb_sb[:, 22 * 128:], in_=wb[:, 22 * 128:])

        # --- PE warm-up ---------------------------------------------------
        def dummy_mms(n):
            """No-dependency filler matmuls: keep the PE busy (HAM at 8/8)
            while DMA-gated real work is pending; near-free when it isn't."""
            for k in range(n):
                ps = psf_pool.tile([128, RPT * HP], F32, tag="psf")
                nc.tensor.matmul(ps[:], lhsT=warm[:, 0:128], rhs=warm[:],
                                 start=True, stop=True)

        dummy_mms(N_WARMUP)

        # xhat: frequency-basis transform of the interior pixels only; the
        # padded borders of x are zero so their transform is zero - memset
        # the xhat border regions once instead of computing them.
        xhat = []
        for c in range(MCH):
            xh_t = xh_pool.tile([128, HP * HP], BF16, tag=f"xh{c}")
            xhat.append(xh_t)
        for c in range(MCH):
            xhv = xhat[c][:].rearrange("p (h w) -> p h w", h=HP)
            nc.vector.memset(xhat[c][:, 0:HP], 0)                # row 0
            nc.vector.memset(xhat[c][:, (HP - 1) * HP:], 0)      # row 57
            nc.vector.memset(xhv[:, 1:HP - 1, 0:1], 0)           # left col
            nc.vector.memset(xhv[:, 1:HP - 1, HP - 1:HP], 0)     # right col
        # interior-row ranges per fwd tile: 7 tiles of 8 rows
        fwd_rows = [(1 + it * RPT, 1 + (it + 1) * RPT) for it in range(7)]

        def fwd_tile(it):
            """Transform interior pixel rows [r0, r1), interior cols."""
            r0, r1 = fwd_rows[it]
            npx = (r1 - r0) * W
            for c in range(MCH):
                ps = psf_pool.tile([128, RPT * HP], F32, tag="psf")
                for i in range(MCH):
                    xv = xbt[i][:].rearrange("p (h w) -> p h w", h=HP)
                    rhs = xv[:, r0:r1, 1:HP - 1]
                    nc.tensor.matmul(
                        ps[:, :npx], lhsT=blk(FWD_BLK(i, c)), rhs=rhs,
                        start=(i == 0), stop=(i == MCH - 1),
                    )
                xho = xhat[c][:].rearrange("p (h w) -> p h w", h=HP)
                nc.vector.tensor_copy(
                    xho[:, r0:r1, 1:HP - 1], ps[:, :npx]
                )

        # out viewed as [p(128), m(2), pix]: c = m*128 + p
        out_v = out.rearrange("(m p) h w -> p m (h w)", m=MCH)

        def conv_inv(row0, nrows, ob_t, ship):
            """Freq conv + inverse transform for output rows [row0, +nrows).

            ship=True DMAs each chunk as soon as its bias-add lands (used
            for the final half-tiles so the writeback drains early).
            """
            npx = nrows * W
            px0 = row0 * W
            ohat = []
            for c in range(MCH):
                ps = psc_pool.tile([128, NT], F32, tag="psc")
                n_mm = 0
                for kh in range(KK):
                    for kw in range(KK):
                        pos = kh * KK + kw
                        xhv = xhat[c][:].rearrange("p (h w) -> p h w", h=HP)
                        rhs = xhv[
                            :, row0 + kh: row0 + kh + nrows, kw: kw + W
                        ]
                        nc.tensor.matmul(
                            ps[:, :npx], lhsT=blk(CONV_BLK(pos, c)), rhs=rhs,
                            start=(n_mm == 0), stop=(n_mm == KK * KK - 1),
                        )
                        n_mm += 1
                oh = oh_pool.tile([128, NT], BF16, tag="oh")
                # split the PSUM->SBUF casts between DVE and ACT so neither
                # becomes the secondary bottleneck
                if c == 0:
                    nc.vector.tensor_copy(oh[:, :npx], ps[:, :npx])
                else:
                    nc.scalar.activation(
                        oh[:, :npx], ps[:, :npx],
                        mybir.ActivationFunctionType.Identity,
                    )
                ohat.append(oh)
            for m in range(MCH):
                ps = psi_pool.tile([128, NT], F32, tag="psi")
                for k in range(MCH):
                    nc.tensor.matmul(
                        ps[:, :npx], lhsT=blk(INV_BLK(k, m)),
                        rhs=ohat[k][:, :npx],
                        start=(k == 0), stop=(k == MCH - 1),
                    )
                nc.scalar.activation(
                    ob_t[:, m, :npx], ps[:, :npx],
                    mybir.ActivationFunctionType.Identity,
                    bias=bias_sb[:, m: m + 1],
                )
                if ship:
                    dma_eng = nc.gpsimd if m == 0 else nc.sync
                    dma_eng.dma_start(
                        out=out_v[:, m, px0:px0 + npx], in_=ob_t[:, m, :npx]
                    )

        # Interleave: fwd runs a few tiles ahead of conv (conv tile nt
        # reads padded xhat rows [nt*8, nt*8+9] = fwd tiles nt and nt+1);
        # the deep lead keeps the PE fed while conv weights stream in.
        for it in range(len(fwd_rows)):
            fwd_tile(it)
            if it in (2, 4):
                dummy_mms(2)
        dummy_mms(3)
        for nt in range(N_TILES):
            if nt < N_TILES - 1:
                ob = ob_pool.tile([128, MCH, NT], F32, tag="ob")
                conv_inv(nt * RPT, RPT, ob, ship=False)
                # alternate output queues so neither backs up at the end
                dma_eng = nc.gpsimd if nt % 2 == 0 else nc.sync
                dma_eng.dma_start(out=out_v[:, :, ts(nt, NT)], in_=ob[:])
            else:
                # last tile as two half-tiles so the writeback drains early
                for r0 in (nt * RPT, nt * RPT + RPT // 2):
                    obh = ob_pool.tile([128, MCH, NT], F32, tag="ob")
                    conv_inv(r0, RPT // 2, obh, ship=True)


def _build_nc():
    nc = bacc.Bacc("TRN2", target_bir_lowering=False, debug=False)
    x = nc.dram_tensor("x", [C, HP, HP], BF16, kind="ExternalInput").ap()
    wb = nc.dram_tensor("wb", [128, N_BLKS * 128], BF16,
                        kind="ExternalInput").ap()
    bias = nc.dram_tensor("bias", [128, MCH], F32, kind="ExternalInput").ap()
    out = nc.dram_tensor("out", [C, H, W], F32, kind="ExternalOutput").ap()
    with tile.TileContext(nc) as tc:
        _kernel_body(tc, x, wb, bias, out)
    nc.compile()
    return nc


def kernel(x: np.ndarray, w: np.ndarray, b: np.ndarray) -> np.ndarray:
    global LAST_RESULT
    xp = np.pad(np.asarray(x, np.float32), ((0, 0), (0, 0), (1, 1), (1, 1)))
    xp = np.ascontiguousarray(xp.astype(ml_dtypes.bfloat16))
    wb = _pack_weights(np.asarray(w, np.float32))
    b = np.ascontiguousarray(np.asarray(b, np.float32).reshape(MCH, 128).T)

    nc = _build_nc()
    in_maps = [{"x": xp[i], "wb": wb, "bias": b} for i in range(N_CORES)]
    trace = bool(int(os.environ.get("KERNEL_PROFILE", "0")))
    res = None
    last_err = None
    for attempt in range(3):
        try:
            res = run_bass_kernel_spmd(
                nc,
                in_maps,
                core_ids=list(range(N_CORES)),
                trace=trace,
            )
            break
        except Exception as e:  # transient device wedge -> retry
            last_err = e
    if res is None:
        raise last_err
    LAST_RESULT = res
    return np.stack([res.results[i]["out"] for i in range(N_CORES)], axis=0)


# revision 17
# speedup vs baseline: 1.0190x; 1.0190x over previous
"""BCM_Conv2d_fft kernel for Trainium2 (8 NeuronCores, batch-parallel).

The reference is a block-circulant 3x3 conv computed via per-block
rfft/irfft over the channel-block axis (block size 8). Per-frequency the
block products are independent, so in a real-DFT channel basis the
256->256 channel mixing matrix of each conv tap is block-diagonal with
frequency groups {f0:32, f4:32, f1:64, f2:64, f3:64}. Grouping
{f0,f4,f1} -> chunk0 and {f2,f3} -> chunk1 makes every tap's mixing
matrix chunk-diagonal: the conv needs 9 matmuls per output tile per
chunk instead of 18 - half the direct-conv PE work.

Device pipeline per core (one image):
  1. fwd:  xhat = A @ x      per pixel    (A = real-DFT, freq-major rows)
  2. conv: ohat = sum_pos M_pos @ shift(xhat)   (chunk-diagonal M)
  3. inv:  out  = Ainv @ ohat + b

All matmuls run in bf16 (measured rel err ~3.1e-3, limit 2e-2): fp32
weights can't use fast-weight-load, so each fp32 LDWEIGHTS takes ~225ns
and gates the matmul cadence at ~252ns, while bf16 weights hide under
the ~190ns N=448 stream. x is padded AND cast to bf16 on the host,
halving its DMA bytes; xhat/ohat are cast to bf16 in the PSUM->SBUF
copies that must happen anyway. The fwd transform covers interior
pixels only (padded borders are zero: their xhat rows/cols are memset).

The first ~10us are DMA-bound (x + weights stream in at ~0.3MB/us over
three queues), so the tensor engine is fed no-dependency dummy matmuls
on a zeroed tile - an initial burst plus fillers between fwd tiles -
which bridge data gaps and keep the HAM clock-gate released (2.4GHz
instead of the 1.2GHz cold state). DMA issue order gives each queue its
first-needed bytes first (x split into 5 row-pieces per chunk, conv
weights into tap-order pieces). All 7 fwd tiles are emitted before the
first conv tile so a weight-blocked conv can never starve ready fwd
work in the tensor FIFO. Outputs alternate between two DMA queues; the
last tile runs as two half-tiles so the final writeback drains early.

Sharding: batch B=8 -> one image per core.
"""

import os

import ml_dtypes
import numpy as np

import concourse.bacc as bacc
import concourse.mybir as mybir
import concourse.tile as tile
from concourse.bass import ts
from concourse.bass_utils import run_bass_kernel_spmd

N_CORES = 8
C = 256
H = W = 56
HP = H + 2
KK = 3
BS = 8
L = H * W
RPT = 8                  # output rows per tile
NT = RPT * W             # 448 pixels per tile
N_TILES = L // NT        # 7
MCH = C // 128           # 2 channel chunks

F32 = mybir.dt.float32
BF16 = mybir.dt.bfloat16

# weight block column indices in the packed wb tensor [128, 26*128] bf16
FWD_BLK = lambda i, c: i * MCH + c            # i = in chunk, c = out chunk
CONV_BLK = lambda pos, c: 4 + pos * MCH + c
INV_BLK = lambda k, m: 22 + k * MCH + m
N_BLKS = 26
N_WARMUP = 10            # dummy matmuls to release the HAM clock gate

LAST_RESULT = None


def _freq_matrices(w: np.ndarray):
    """Build A [256,256], Ms (9x [256,256] chunk-diag), Ainv from w."""
    F = np.zeros((8, 8))
    FI = np.fft.rfft(np.eye(8), axis=-1)
    F[0] = FI[:, 0].real
    F[1], F[2] = FI[:, 1].real, FI[:, 1].imag
    F[3], F[4] = FI[:, 2].real, FI[:, 2].imag
    F[5], F[6] = FI[:, 3].real, FI[:, 3].imag
    F[7] = FI[:, 4].real

    def fm(bk, comp):
        if comp == 0:
            return bk
        if comp == 7:
            return 32 + bk
        if comp in (1, 2):
            return 64 + 2 * bk + (comp - 1)
        if comp in (3, 4):
            return 128 + 2 * bk + (comp - 3)
        return 192 + 2 * bk + (comp - 5)

    A = np.zeros((256, 256))
    for bk in range(32):
        for comp in range(8):
            A[fm(bk, comp), bk * 8:(bk + 1) * 8] = F[comp]
    Ainv = np.linalg.inv(A)

    wf = np.fft.rfft(w.astype(np.float64), axis=-1)  # [32, 288, 5]
    Ms = []
    for pos in range(9):
        M = np.zeros((256, 256))
        for pb in range(32):
            for kb in range(32):
                kc = pos * 32 + kb
                M[fm(pb, 0), fm(kb, 0)] += wf[pb, kc, 0].real
                M[fm(pb, 7), fm(kb, 7)] += wf[pb, kc, 4].real
                for fi in range(3):
                    re_i, im_i = 1 + 2 * fi, 2 + 2 * fi
                    Wr, Wi = wf[pb, kc, fi + 1].real, wf[pb, kc, fi + 1].imag
                    M[fm(pb, re_i), fm(kb, re_i)] += Wr
                    M[fm(pb, re_i), fm(kb, im_i)] += -Wi
                    M[fm(pb, im_i), fm(kb, re_i)] += Wi
                    M[fm(pb, im_i), fm(kb, im_i)] += Wr
        Ms.append(M)
    return A, Ms, Ainv


def _pack_weights(w: np.ndarray) -> np.ndarray:
    """-> [128, 26*128] bf16: lhsT blocks for fwd, conv, inv stages."""
    A, Ms, Ainv = _freq_matrices(w)
    wb = np.zeros((128, N_BLKS * 128), np.float32)

    sl = lambda i: slice(i * 128, (i + 1) * 128)
    for i in range(MCH):
        for c in range(MCH):
            wb[:, sl(FWD_BLK(i, c))] = A[sl(c), sl(i)].T
    for pos in range(9):
        for c in range(MCH):
            wb[:, sl(CONV_BLK(pos, c))] = Ms[pos][sl(c), sl(c)].T
    for k in range(MCH):
        for m in range(MCH):
            wb[:, sl(INV_BLK(k, m))] = Ainv[sl(m), sl(k)].T
    return wb.astype(ml_dtypes.bfloat16)


def _kernel_body(tc, x, wb, bias, out):
    nc = tc.nc
    with (
        tc.tile_pool(name="const", bufs=1) as const_pool,
        tc.tile_pool(name="xb", bufs=1) as xb_pool,
        tc.tile_pool(name="xh", bufs=1) as xh_pool,
        tc.tile_pool(name="oh", bufs=6) as oh_pool,
        tc.tile_pool(name="ob", bufs=4) as ob_pool,
        tc.tile_pool(name="psf", bufs=3, space="PSUM") as psf_pool,
        tc.tile_pool(name="psc", bufs=3, space="PSUM") as psc_pool,
        tc.tile_pool(name="psi", bufs=2, space="PSUM") as psi_pool,
    ):
        # PE warm-up source: zeroed first thing (cheap, ~115ns) so the
        # dummy matmuls can start the moment the PE preamble ends
        warm = const_pool.tile([128, RPT * HP], BF16)
        nc.gpsimd.memset(warm[:], 0)

        # --- DMA schedule -------------------------------------------------
        # Three issuing engines feed three hardware queues; each queue gets
        # its first-needed bytes first:
        #   sync:   x c0 rows 0-8, bias, x c0 rest, conv weights pos 6-8
        #   gpsimd: x c1 rows 0-8, x c1 rest, inv weights (outputs later)
        #   scalar: fwd weights, conv weights pos 0-5
        wb_sb = const_pool.tile([128, N_BLKS * 128], BF16)
        blk = lambda idx: wb_sb[:, ts(idx, 128)]

        xbt = []
        for i in range(MCH):
            xb_t = xb_pool.tile([128, HP * HP], BF16, tag=f"xb{i}")
            xbt.append(xb_t)
        bias_sb = const_pool.tile([128, MCH], F32)

        row_splits = [0, 10, 20, 30, 44, HP]
        eng = [nc.sync, nc.gpsimd]
        for i in range(MCH):
            r0, r1 = row_splits[0], row_splits[1]
            eng[i].dma_start(
                out=xbt[i][:, r0 * HP:r1 * HP],
                in_=x[ts(i, 128), r0:r1, :].rearrange("p h w -> p (h w)"),
            )
        nc.scalar.dma_start(out=wb_sb[:, 0:4 * 128], in_=wb[:, 0:4 * 128])
        nc.scalar.dma_start(out=bias_sb[:], in_=bias[:, :])
        for r0, r1 in zip(row_splits[1:-1], row_splits[2:]):
            for i in range(MCH):
                eng[i].dma_start(
                    out=xbt[i][:, r0 * HP:r1 * HP],
                    in_=x[ts(i, 128), r0:r1, :].rearrange("p h w -> p (h w)"),
                )
        nc.scalar.dma_start(out=wb_sb[:, 4 * 128:10 * 128],
                            in_=wb[:, 4 * 128:10 * 128])
        nc.scalar.dma_start(out=wb_sb[:, 10 * 128:16 * 128],
                            in_=wb[:, 10 * 128:16 * 128])
        nc.scalar.dma_start(out=wb_sb[:, 16 * 128:22 * 128],
                            in_=wb[:, 16 * 128:22 * 128])
        nc.scalar.dma_start(out=wb_sb[:, 22 * 128:], in_=wb[:, 22 * 128:])

        # --- PE warm-up ---------------------------------------------------
        def dummy_mms(n):
            """No-dependency filler matmuls: keep the PE busy (HAM at 8/8)
            while DMA-gated real work is pending; near-free when it isn't."""
            for k in range(n):
                ps = psf_pool.tile([128, RPT * HP], F32, tag="psf")
                nc.tensor.matmul(ps[:], lhsT=warm[:, 0:128], rhs=warm[:],
                                 start=True, stop=True)

        dummy_mms(N_WARMUP)

        # xhat: frequency-basis transform of the interior pixels only; the
        # padded borders of x are zero so their transform is zero - memset
        # the xhat border regions once instead of computing them.
        xhat = []
        for c in range(MCH):
            xh_t = xh_pool.tile([128, HP * HP], BF16, tag=f"xh{c}")
            xhat.append(xh_t)
        for c in range(MCH):
            xhv = xhat[c][:].rearrange("p (h w) -> p h w", h=HP)
            nc.vector.memset(xhat[c][:, 0:HP], 0)                # row 0
            nc.vector.memset(xhat[c][:, (HP - 1) * HP:], 0)      # row 57
            nc.vector.memset(xhv[:, 1:HP - 1, 0:1], 0)           # left col
            nc.vector.memset(xhv[:, 1:HP - 1, HP - 1:HP], 0)     # right col
        # interior-row ranges per fwd tile: 7 tiles of 8 rows
        fwd_rows = [(1 + it * RPT, 1 + (it + 1) * RPT) for it in range(7)]

        def fwd_tile(it):
            """Transform interior pixel rows [r0, r1), interior cols."""
            r0, r1 = fwd_rows[it]
            npx = (r1 - r0) * W
            for c in range(MCH):
                ps = psf_pool.tile([128, RPT * HP], F32, tag="psf")
                for i in range(MCH):
                    xv = xbt[i][:].rearrange("p (h w) -> p h w", h=HP)
                    rhs = xv[:, r0:r1, 1:HP - 1]
                    nc.tensor.matmul(
                        ps[:, :npx], lhsT=blk(FWD_BLK(i, c)), rhs=rhs,
                        start=(i == 0), stop=(i == MCH - 1),
                    )
                xho = xhat[c][:].rearrange("p (h w) -> p h w", h=HP)
                nc.vector.tensor_copy(
                    xho[:, r0:r1, 1:HP - 1], ps[:, :npx]
                )

        # out viewed as [p(128), m(2), pix]: c = m*128 + p
        out_v = out.rearrange("(m p) h w -> p m (h w)", m=MCH)

        ship_eng = [nc.gpsimd, nc.sync, nc.scalar]
        ship_state = [0]

        def conv_inv(row0, nrows, ob_t):
            """Freq conv + inverse transform for output rows [row0, +nrows).

            Each chunk is DMA'd to HBM as soon as its bias-add lands, on a
            rotating queue, so no single queue backs up near the end.
            """
            npx = nrows * W
            px0 = row0 * W
            ohat = []
            for c in range(MCH):
                ps = psc_pool.tile([128, NT], F32, tag="psc")
                n_mm = 0
                for kh in range(KK):
                    for kw in range(KK):
                        pos = kh * KK + kw
                        xhv = xhat[c][:].rearrange("p (h w) -> p h w", h=HP)
                        rhs = xhv[
                            :, row0 + kh: row0 + kh + nrows, kw: kw + W
                        ]
                        nc.tensor.matmul(
                            ps[:, :npx], lhsT=blk(CONV_BLK(pos, c)), rhs=rhs,
                            start=(n_mm == 0), stop=(n_mm == KK * KK - 1),
                        )
                        n_mm += 1
                oh = oh_pool.tile([128, NT], BF16, tag="oh")
                # split the PSUM->SBUF casts between DVE and ACT so neither
                # becomes the secondary bottleneck
                if c == 0:
                    nc.vector.tensor_copy(oh[:, :npx], ps[:, :npx])
                else:
                    nc.scalar.activation(
                        oh[:, :npx], ps[:, :npx],
                        mybir.ActivationFunctionType.Identity,
                    )
                ohat.append(oh)
            for m in range(MCH):
                ps = psi_pool.tile([128, NT], F32, tag="psi")
                for k in range(MCH):
                    nc.tensor.matmul(
                        ps[:, :npx], lhsT=blk(INV_BLK(k, m)),
                        rhs=ohat[k][:, :npx],
                        start=(k == 0), stop=(k == MCH - 1),
                    )
                nc.scalar.activation(
                    ob_t[:, m, :npx], ps[:, :npx],
                    mybir.ActivationFunctionType.Identity,
                    bias=bias_sb[:, m: m + 1],
                )
                dma_eng = ship_eng[ship_state[0] % len(ship_eng)]
                ship_state[0] += 1
                dma_eng.dma_start(
                    out=out_v[:, m, px0:px0 + npx], in_=ob_t[:, m, :npx]
                )

        # Interleave: fwd runs a few tiles ahead of conv (conv tile nt
        # reads padded xhat rows [nt*8, nt*8+9] = fwd tiles nt and nt+1);
        # the deep lead keeps the PE fed while conv weights stream in.
        for it in range(len(fwd_rows)):
            fwd_tile(it)
            if it in (2, 4):
                dummy_mms(2)
        dummy_mms(3)
        for nt in range(N_TILES):
            if nt < N_TILES - 1:
                ob = ob_pool.tile([128, MCH, NT], F32, tag="ob")
                conv_inv(nt * RPT, RPT, ob)
            else:
                # last tile as two half-tiles so the writeback drains early
                for r0 in (nt * RPT, nt * RPT + RPT // 2):
                    obh = ob_pool.tile([128, MCH, NT], F32, tag="ob")
                    conv_inv(r0, RPT // 2, obh)


def _build_nc():
    nc = bacc.Bacc("TRN2", target_bir_lowering=False, debug=False)
    x = nc.dram_tensor("x", [C, HP, HP], BF16, kind="ExternalInput").ap()
    wb = nc.dram_tensor("wb", [128, N_BLKS * 128], BF16,
                        kind="ExternalInput").ap()
    bias = nc.dram_tensor("bias", [128, MCH], F32, kind="ExternalInput").ap()
    out = nc.dram_tensor("out", [C, H, W], F32, kind="ExternalOutput").ap()
    with tile.TileContext(nc) as tc:
        _kernel_body(tc, x, wb, bias, out)
    nc.compile()
    return nc


def kernel(x: np.ndarray, w: np.ndarray, b: np.ndarray) -> np.ndarray:
    global LAST_RESULT
    xp = np.pad(np.asarray(x, np.float32), ((0, 0), (0, 0), (1, 1), (1, 1)))
    xp = np.ascontiguousarray(xp.astype(ml_dtypes.bfloat16))
    wb = _pack_weights(np.asarray(w, np.float32))
    b = np.ascontiguousarray(np.asarray(b, np.float32).reshape(MCH, 128).T)

    nc = _build_nc()
    in_maps = [{"x": xp[i], "wb": wb, "bias": b} for i in range(N_CORES)]
    trace = bool(int(os.environ.get("KERNEL_PROFILE", "0")))
    res = None
    last_err = None
    for attempt in range(3):
        try:
            res = run_bass_kernel_spmd(
                nc,
                in_maps,
                core_ids=list(range(N_CORES)),
                trace=trace,
            )
            break
        except Exception as e:  # transient device wedge -> retry
            last_err = e
    if res is None:
        raise last_err
    LAST_RESULT = res
    return np.stack([res.results[i]["out"] for i in range(N_CORES)], axis=0)
